# revision 1
# baseline (speedup 1.0000x reference)
"""MeshConv (Chebyshev graph conv, K=6) Trainium2 kernel, 8 NeuronCores.

Strategy: vertex (dst-row) sharding across the 8 cores with 8-batch "tokens"
(one token = all 8 batches' 64 features of one vertex = 512 values, bf16 for
gathers).  Per Chebyshev step: AllGather the bf16 token array, dma_gather
per-edge source tokens into a fixed slot grid, multiply-accumulate per
128-row dst tile on the TensorEngine with host-built [slots x rows] value
patterns (edge weights live in the stationary operand), then a fused DVE
recurrence update in fp32.  The dense projection folds W into block-diagonal
per-batch-pair matrices applied to xbar-transposed bf16 activations.
"""
import os
import sys
import time

sys.path.insert(0, '/opt/trn_rl_repo')

import numpy as np
import ml_dtypes

import concourse.bass as bass
import concourse.bacc as bacc
import concourse.mybir as mybir
import concourse.tile as tile_mod
from concourse.tile import TileContext
from concourse.bass_utils import run_bass_kernel_spmd

# ---------------------------------------------------------------- constants
B, FIN, K, FOUT = 8, 64, 6, 64
NCORE = 8
TOK = B * FIN              # 512 values per vertex token

# walrus in this environment accepts only 1 sync-wait per CTRL instruction:
# spread the Tile tail-drain's waits across preceding nops.
def _patched_drain_and_barrier(self, tick_clock, wait_clock):
    nop0 = self.nc.sync.nop(nofuse=True)
    wait_clock.add_sem_waits(nop0.ins, tile_mod.ScopedClock({None: tick_clock.global_clock}))
    si = nop0.ins.sync_info
    waits = list(si.on_wait) if si and si.on_wait else []
    if len(waits) > 1:
        si.on_wait = waits[:1]
        rest = waits[1:]
        while rest:
            n = self.nc.sync.nop(nofuse=True)
            nsi = n.ins.sync_info
            if nsi is None:
                n.ins.sync_info = mybir.SyncInfo(on_wait=rest[:1], on_update=[])
            else:
                nsi.on_wait = rest[:1]
            rest = rest[1:]
    self.nc.sync.drain()
    self.nc.all_engine_barrier()
    assert self.sems is not None
    popped = self.nc._tile_sem_poison_stack.pop()
    assert popped is self._sem_poison
    self.nc.clear_and_free_semaphores(list(self.sems.allocated().values()))
    self.nc.all_engine_barrier()


tile_mod.TileContext._drain_and_barrier = _patched_drain_and_barrier


class Cfg:
    """Geometry of the slot grid.  Everything derives from (M, CPT_A, CPT_B)."""

    def __init__(self, M, ntile_core, cpt_a, cpt_b, ga_call, gb_call):
        self.M = M                           # real vertex count
        self.NTILE_CORE = ntile_core         # 128-row dst tiles per core
        self.SLICE = 128 * ntile_core        # rows per core
        self.MPAD = NCORE * self.SLICE       # padded vertex positions
        self.NTILE = NCORE * ntile_core
        self.CPT_A = cpt_a                   # A-chunks per tile
        self.CPT_B = cpt_b                   # B-chunks per tile
        self.CPT = cpt_a + cpt_b
        self.NCH_A = cpt_a * ntile_core      # A chunks per core
        self.NCH_B = cpt_b * ntile_core
        self.NCH = self.CPT * ntile_core
        self.NIDX_A = self.NCH_A * 128
        self.NIDX_B = self.NCH_B * 128
        self.GA_CALL = ga_call               # idxs per A gather call
        self.GB_CALL = gb_call
        # int16 index split: call A covers positions [0, 32768); call B uses
        # base ASPLIT-BSHIFT... B base chosen so B indices stay in [0, 32768).
        self.ASPLIT = min(32768, self.MPAD)  # positions < ASPLIT reachable by A
        self.BBASE = max(0, self.MPAD - 32768)  # B call base row
        assert self.MPAD - self.BBASE <= 32768

    def a_calls(self):
        """List of (start_chunk, n_idx) for the A gather calls."""
        out = []
        ch = 0
        while ch * 128 < self.NIDX_A:
            n = min(self.GA_CALL, self.NIDX_A - ch * 128)
            out.append((ch, n))
            ch += n // 128
        return out

    def b_calls(self):
        out = []
        ch = 0
        while ch * 128 < self.NIDX_B:
            n = min(self.GB_CALL, self.NIDX_B - ch * 128)
            out.append((ch, n))
            ch += n // 128
        return out


CFG_FULL = Cfg(M=40000, ntile_core=40, cpt_a=7, cpt_b=2, ga_call=4096, gb_call=2048)


# ---------------------------------------------------------------- host prep
def build_graph_data(cfg, edge_rows, edge_cols, edge_vals):
    """Slot the edge list into the fixed per-tile chunk grid.

    Returns per-core idxA/idxB (wrapped int16), pattern array, and the
    vertex<->position permutation.
    """
    M, MPAD = cfg.M, cfg.MPAD
    er = np.asarray(edge_rows).astype(np.int64)
    ec = np.asarray(edge_cols).astype(np.int64)
    ev = np.asarray(edge_vals).astype(np.float32)
    E = er.shape[0]

    outdeg = np.bincount(ec, minlength=M)
    indeg = np.bincount(er, minlength=M)

    # Zone split: lowest out-degree vertices go to the B zone (positions >=
    # ASPLIT) so B-only edges per tile stay small.
    nb_real = max(0, MPAD - cfg.ASPLIT - (MPAD - M))  # real vertices in B zone
    na_real = M - nb_real
    order_by_out = np.argsort(outdeg, kind="stable")
    bverts = order_by_out[:nb_real]
    averts = order_by_out[nb_real:]

    ntile_a = cfg.ASPLIT // 128
    ntile_b = (MPAD - cfg.ASPLIT) // 128
    v2pos = np.full(M, -1, np.int64)
    # in-degree balance: sort desc by indeg, round-robin over zone tiles
    a_sorted = averts[np.argsort(-indeg[averts], kind="stable")]
    i = np.arange(na_real)
    v2pos[a_sorted] = 128 * (i % ntile_a) + (i // ntile_a)
    if nb_real:
        b_sorted = bverts[np.argsort(-indeg[bverts], kind="stable")]
        i = np.arange(nb_real)
        assert (i // ntile_b).max() < 128
        v2pos[b_sorted] = cfg.ASPLIT + 128 * (i % ntile_b) + (i // ntile_b)
    assert (v2pos >= 0).all()

    rpos = v2pos[er]
    cpos = v2pos[ec]
    tile = rpos // 128
    rloc = rpos % 128

    # Per tile, split edges between A chunks (src pos < ASPLIT) and B chunks
    # (src pos >= BBASE), respecting capacities.
    capA = cfg.CPT_A * 128
    capB = cfg.CPT_B * 128
    idxA = np.zeros((NCORE, cfg.NIDX_A), np.int16)
    idxB = np.zeros((NCORE, cfg.NIDX_B), np.int16)
    pat = np.zeros((NCORE, cfg.NCH, 128, 128), np.float32)

    order = np.lexsort((cpos, tile))   # group by tile; B-eligible sorted last
    er_s, tile_s, rloc_s, cpos_s, ev_s = er[order], tile[order], rloc[order], cpos[order], ev[order]
    tstart = np.searchsorted(tile_s, np.arange(cfg.NTILE + 1))

    for t in range(cfg.NTILE):
        lo, hi = tstart[t], tstart[t + 1]
        n = hi - lo
        if n > capA + capB:
            raise RuntimeError(f"tile {t} overflow: {n} edges > {capA + capB}")
        cp = cpos_s[lo:hi]
        rl = rloc_s[lo:hi]
        vv = ev_s[lo:hi]
        bmask = cp >= cfg.ASPLIT            # must go to B
        amask = cp < cfg.BBASE              # must go to A
        nB_only = int(bmask.sum())
        if nB_only > capB:
            raise RuntimeError(f"tile {t}: B-only {nB_only} > capB {capB}")
        needB = max(nB_only, n - capA)
        # promote flexible (mid-range) edges to B if A would overflow
        bsel = bmask.copy()
        if needB > nB_only:
            flex = np.flatnonzero(~bmask & ~amask)
            bsel[flex[: needB - nB_only]] = True
        asel = ~bsel
        nA, nB = int(asel.sum()), int(bsel.sum())
        assert nA <= capA and nB <= capB, (t, nA, nB)

        core = t // cfg.NTILE_CORE
        tl = t % cfg.NTILE_CORE
        # A slots
        s = np.arange(nA)
        chA = tl * cfg.CPT_A + s // 128
        slA = s % 128
        idxA[core, chA * 128 + slA] = cp[asel].astype(np.int16)
        pat[core, (tl * cfg.CPT + (s // 128)), slA, rl[asel]] = vv[asel]
        # B slots
        s = np.arange(nB)
        chB = tl * cfg.CPT_B + s // 128
        slB = s % 128
        idxB[core, chB * 128 + slB] = (cp[bsel] - cfg.BBASE).astype(np.int16)
        pat[core, (tl * cfg.CPT + cfg.CPT_A + (s // 128)), slB, rl[bsel]] = vv[bsel]

    def wrap(idx):
        # dma_gather layout: idx i -> partition i%16, free i//16, replicated x8
        n = idx.shape[1]
        a = idx.reshape(NCORE, n // 16, 16).transpose(0, 2, 1)  # [NCORE, 16, n/16]
        return np.tile(a, (1, 8, 1)).copy()

    return {
        "idxA_w": wrap(idxA),
        "idxB_w": wrap(idxB),
        "pat": pat.astype(ml_dtypes.bfloat16),
        "v2pos": v2pos,
    }


def build_w_blocks(W):
    """W [FIN*K, FOUT] -> per-k block-diagonal [128, 128] (2 batches/block)."""
    Wk = np.asarray(W).astype(np.float32).reshape(FIN, K, FOUT)  # [fin, k, fo]
    blocks = np.zeros((K, 128, 128), np.float32)
    for k in range(K):
        blocks[k, 0:64, 0:64] = Wk[:, k, :]
        blocks[k, 64:128, 64:128] = Wk[:, k, :]
    return blocks.astype(ml_dtypes.bfloat16)


def build_x0(cfg, x, v2pos):
    """x [B, M, FIN] -> per-core fp32 token slices [SLICE, TOK] (b-major)."""
    M = cfg.M
    tok = np.zeros((cfg.MPAD, TOK), np.float32)
    xt = np.transpose(np.asarray(x).astype(np.float32), (1, 0, 2)).reshape(M, TOK)
    tok[v2pos] = xt
    return tok.reshape(NCORE, cfg.SLICE, TOK)


# ---------------------------------------------------------------- device IR
def build_nc(cfg, repeat=1):
    nc = bacc.Bacc(None, target_bir_lowering=False, debug=False,
                   dynamic_dma_scratch_size=16384)
    dt = mybir.dt
    S, T = cfg.SLICE, cfg.NTILE_CORE

    x0loc = nc.declare_dram_parameter("x0loc", [S, TOK], dt.float32, isOutput=False)
    idxA = nc.declare_dram_parameter("idxA", [128, cfg.NIDX_A // 16], dt.int16, isOutput=False)
    idxB = nc.declare_dram_parameter("idxB", [128, cfg.NIDX_B // 16], dt.int16, isOutput=False)
    patd = nc.declare_dram_parameter("pat", [cfg.NCH * 128, 128], dt.bfloat16, isOutput=False)
    wblk = nc.declare_dram_parameter("wblk", [K * 128, 128], dt.bfloat16, isOutput=False)
    outp = nc.declare_dram_parameter("outp", [512, S], dt.float32, isOutput=True)

    contrib = [nc.dram_tensor(f"contrib{k}", [S, TOK], dt.bfloat16) for k in range(K)]
    gathered = [nc.dram_tensor(f"gathered{k}", [cfg.MPAD, TOK], dt.bfloat16,
                               addr_space="Shared") for k in range(1, K)]
    xf = [x0loc] + [nc.dram_tensor(f"xf{k}", [S, TOK], dt.float32) for k in range(1, K)]

    a_calls = cfg.a_calls()
    b_calls = cfg.b_calls()
    # map chunk -> (call index, slot-in-call)
    def chunk_map(calls):
        m = {}
        for ci, (ch0, n) in enumerate(calls):
            for j in range(n // 128):
                m[ch0 + j] = (ci, j)
        return m

    amap, bmap = chunk_map(a_calls), chunk_map(b_calls)
    ga_free = max(n // 128 for _, n in a_calls)
    gb_free = max(n // 128 for _, n in b_calls)

    with TileContext(nc) as tc:
        with (
            tc.tile_pool(name="io", bufs=1) as io,
            tc.tile_pool(name="ga", bufs=2) as gap,
            tc.tile_pool(name="gb", bufs=2) as gbp,
            tc.tile_pool(name="patp", bufs=3) as patp,
            tc.tile_pool(name="ev", bufs=3) as evp,
            tc.tile_pool(name="prj", bufs=2) as prjp,
            tc.tile_pool(name="ps", bufs=3, space="PSUM") as psp,
            tc.tile_pool(name="psj", bufs=2, space="PSUM") as psjp,
        ):
            # resident: gather indices + W blocks
            idxA_t = io.tile([128, cfg.NIDX_A // 16], dt.int16)
            nc.sync.dma_start(out=idxA_t[:], in_=idxA[:])
            idxB_t = io.tile([128, cfg.NIDX_B // 16], dt.int16)
            nc.sync.dma_start(out=idxB_t[:], in_=idxB[:])
            w_t = io.tile([128, K, 128], dt.bfloat16)
            nc.sync.dma_start(out=w_t[:], in_=wblk[:].rearrange("(k p) r -> p k r", p=128))

            def projection(k):
                # outp[j*128 + (2b'|fo), r] += sum_fin Wk x_k
                for j in range(4):
                    xT = prjp.tile([128, S], dt.bfloat16, tag="xT")
                    nc.sync.dma_start(out=xT[:], in_=contrib[k][:, j * 128:(j + 1) * 128],
                                      transpose=True)
                    for rc in range(S // 512):
                        pj = psjp.tile([128, 512], dt.float32, tag="pj")
                        nc.tensor.matmul(pj[:], w_t[:, k, :], xT[:, rc * 512:(rc + 1) * 512],
                                         start=True, stop=True)
                        acc = prjp.tile([128, 512], dt.float32, tag="acc")
                        nc.sync.dma_start(out=acc[:], in_=outp[j * 128:(j + 1) * 128,
                                                              rc * 512:(rc + 1) * 512])
                        acc2 = prjp.tile([128, 512], dt.float32, tag="acc2")
                        nc.vector.tensor_add(acc2[:], acc[:], pj[:])
                        nc.sync.dma_start(out=outp[j * 128:(j + 1) * 128,
                                                   rc * 512:(rc + 1) * 512], in_=acc2[:])

            def stage0():
                for g in range(0, T, 2):
                    nt = min(2, T - g)
                    t0 = evp.tile([128, nt, TOK], dt.float32, tag="s0f")
                    nc.sync.dma_start(out=t0[:], in_=x0loc[:].rearrange(
                        "(a p) f -> p a f", p=128)[:, g:g + nt, :])
                    t0b = evp.tile([128, nt, TOK], dt.bfloat16, tag="s0b")
                    nc.vector.tensor_copy(t0b[:], t0[:])
                    nc.sync.dma_start(out=contrib[0][:].rearrange(
                        "(a p) f -> p a f", p=128)[:, g:g + nt, :], in_=t0b[:])

            def cheb_step(k, gk):
                gk = gathered[k - 1]
                nc.gpsimd.collective_compute(
                    "AllGather", mybir.AluOpType.bypass,
                    replica_groups=[list(range(NCORE))],
                    ins=[contrib[k - 1][:]], outs=[gk[:]],
                )
                GA, GB = [], []
                for (ch0, n) in a_calls:
                    g = gap.tile([128, ga_free, TOK], dt.bfloat16, tag="ga")
                    nc.gpsimd.dma_gather(
                        out_ap=g[:, : n // 128, :], in_ap=gk[0:cfg.ASPLIT, :],
                        idxs_ap=idxA_t[:, ch0 * 8: ch0 * 8 + n // 16],
                        num_idxs=n, num_idxs_reg=n, elem_size=TOK,
                        single_packet=False)
                    GA.append(g)
                for (ch0, n) in b_calls:
                    g = gbp.tile([128, gb_free, TOK], dt.bfloat16, tag="gb")
                    nc.gpsimd.dma_gather(
                        out_ap=g[:, : n // 128, :], in_ap=gk[cfg.BBASE:, :],
                        idxs_ap=idxB_t[:, ch0 * 8: ch0 * 8 + n // 16],
                        num_idxs=n, num_idxs_reg=n, elem_size=TOK,
                        single_packet=False)
                    GB.append(g)

                for tl in range(T):
                    pt = patp.tile([128, cfg.CPT, 128], dt.bfloat16, tag="pat")
                    nc.sync.dma_start(out=pt[:], in_=patd[:].rearrange(
                        "(c s) r -> s c r", s=128)[:, tl * cfg.CPT:(tl + 1) * cfg.CPT, :])
                    ps = psp.tile([128, TOK], dt.float32, tag="ps")
                    for j in range(cfg.CPT_A):
                        ci, sl = amap[tl * cfg.CPT_A + j]
                        nc.tensor.matmul(ps[:], pt[:, j, :], GA[ci][:, sl, :],
                                         start=(j == 0), stop=False)
                    for j in range(cfg.CPT_B):
                        ci, sl = bmap[tl * cfg.CPT_B + j]
                        nc.tensor.matmul(ps[:], pt[:, cfg.CPT_A + j, :], GB[ci][:, sl, :],
                                         start=False, stop=(j == cfg.CPT_B - 1))
                    # recurrence: k=1: x1 = ps - x0 ; k>1: xk = 2 ps - 2 x_{k-1} - x_{k-2}
                    xprev = evp.tile([128, TOK], dt.float32, tag="xprev")
                    nc.sync.dma_start(out=xprev[:], in_=xf[k - 1][tl * 128:(tl + 1) * 128, :])
                    xk_t = evp.tile([128, TOK], dt.float32, tag="xk")
                    if k == 1:
                        nc.vector.scalar_tensor_tensor(
                            xk_t[:], ps[:], 1.0, xprev[:],
                            op0=mybir.AluOpType.mult, op1=mybir.AluOpType.subtract)
                    else:
                        xpp = evp.tile([128, TOK], dt.float32, tag="xpp")
                        nc.sync.dma_start(out=xpp[:], in_=xf[k - 2][tl * 128:(tl + 1) * 128, :])
                        tmp = evp.tile([128, TOK], dt.float32, tag="tmp")
                        nc.vector.scalar_tensor_tensor(
                            tmp[:], xprev[:], 2.0, xpp[:],
                            op0=mybir.AluOpType.mult, op1=mybir.AluOpType.add)
                        nc.vector.scalar_tensor_tensor(
                            xk_t[:], ps[:], 2.0, tmp[:],
                            op0=mybir.AluOpType.mult, op1=mybir.AluOpType.subtract)
                    nc.sync.dma_start(out=xf[k][tl * 128:(tl + 1) * 128, :], in_=xk_t[:])
                    xkb = evp.tile([128, TOK], dt.bfloat16, tag="xkb")
                    nc.vector.tensor_copy(xkb[:], xk_t[:])
                    nc.sync.dma_start(out=contrib[k][tl * 128:(tl + 1) * 128, :], in_=xkb[:])
                projection(k)

            for _rep in range(repeat):
                stage0()
                projection(0)
                for k in range(1, K):
                    cheb_step(k, None)

    nc.finalize()
    return nc


_NC_CACHE = {}


def get_nc(cfg, repeat=1):
    key = (cfg.M, cfg.NTILE_CORE, cfg.CPT_A, cfg.CPT_B, repeat)
    if key not in _NC_CACHE:
        _NC_CACHE[key] = build_nc(cfg, repeat)
    return _NC_CACHE[key]


# ---------------------------------------------------------------- entry
def run(cfg, x, edge_vals, W, edge_rows, edge_cols, trace=False):
    g = build_graph_data(cfg, edge_rows, edge_cols, edge_vals)
    x0 = build_x0(cfg, x, g["v2pos"])
    wb = build_w_blocks(W)
    nc = get_nc(cfg)
    in_maps = []
    for c in range(NCORE):
        in_maps.append({
            "x0loc": x0[c],
            "idxA": g["idxA_w"][c],
            "idxB": g["idxB_w"][c],
            "pat": np.ascontiguousarray(g["pat"][c].reshape(cfg.NCH * 128, 128)),
            "wblk": np.ascontiguousarray(wb.reshape(K * 128, 128)),
        })
    res = run_bass_kernel_spmd(nc, in_maps, core_ids=list(range(NCORE)), trace=trace)
    # assemble: outp [512, SLICE]; row j*128 + b_loc*64 + fo with b = 2j + b_loc
    out_all = np.stack([res.results[c]["outp"] for c in range(NCORE)], 0)  # [NC, 512, S]
    out_all = out_all.reshape(NCORE, 4, 2, FOUT, cfg.SLICE)
    out_pos = out_all.transpose(1, 2, 0, 4, 3).reshape(B, cfg.MPAD, FOUT)
    out = np.empty((B, cfg.M, FOUT), np.float32)
    out[:] = out_pos[:, g["v2pos"], :]
    return out, res


def kernel(**inputs):
    out, _ = run(CFG_FULL, inputs["x"], inputs["edge_vals"], inputs["W"],
                 inputs["edge_rows"], inputs["edge_cols"])
    return out

